# revision 4
# baseline (speedup 1.0000x reference)
"""BoundaryMaxPooling Trainium2 kernel, v3.

Reference (B=16, C2=512, T=Tf=126): window maxes over feature[:, :, j]
with per-t windows derived from segments[0] (two families: start/end).

Sharding: family-per-core. Cores 0-3 compute the START half (channels
0:256) of batches [4i, 4i+4); cores 4-7 the END half. Each core holds a
[j=126, c'=1024] bf16 layout (c' = local_batch*256 + channel) so one
window family covers all 1024 columns.

Device algorithm per core:
  ladder  L1=F, L2=max(F,F+1) (host ships F||F+1 halves on the two fast
          HWDGE queues), L{2s}=max(Ls, Ls shifted s) for s=2..32 (PE band
          matmul to PSUM + DVE max, half-column pipelined),
  gathers 2 lookups per window (rows lo, hi-s of level s=2^floor(log2 L))
          as one-hot matmuls into two PSUM accumulators; each ladder
          gap is filled with that level's own gathers so only the s=64
          gathers remain after the ladder,
  merge   out = max(acc0, acc1) via DVE copy + DVE max (no ACT ops, so
          the Scalar queue has no act-table load ahead of its DMAs),
  out     [126, 1024] bf16 -> host converts/reassembles to fp32.

DMA layout (deadline-ordered): fa/fb carry the features, the +1-shifted
copies, the first two shift bands and the s<=2 one-hots; three staged
side loads carry the later bands/one-hots on Sync/Scalar/GpSimd.

All index math is host-side (depends only on segments[0], data-
independent); all feature-dependent compute runs on device.
"""

import os
import sys

import numpy as np

if os.path.isdir("/opt/trn_rl_repo") and "/opt/trn_rl_repo" not in sys.path:
    sys.path.insert(0, "/opt/trn_rl_repo")

import concourse.bass as bass  # noqa: E402
from concourse import bacc, mybir, tile  # noqa: E402
from concourse.bass_utils import run_bass_kernel_spmd  # noqa: E402

B, C2, T = 16, 512, 126
C = C2 // 2  # 256
NCORES = 8
BPC = 4  # batches per core (family sharding: 4 cores per family)
W = BPC * C  # 1024 columns per core
H = 512  # half width

SIZES = [1, 2, 4, 8, 16, 32, 64]
ROWS = {s: 127 - s for s in SIZES}

F32 = mybir.dt.float32
BF16 = mybir.dt.bfloat16

_CACHE = {}
TRACE = False
LAST_RESULTS = None

# fa: F_h0 | F1_h0 | band2                          -> [T, 2H + T]
# fb: F_h1 | F1_h1 | band4                          -> [T, 2H + T]
# ga (Sync 2nd):   g(1,*) | g(2,*) | bands{16,32,64} -> [T, 7T]
# gb (Scalar 2nd): g(4,*) | g(8,*)                   -> [T, 4T]
# gc (GpSimd):     g(16,*) | g(32,*) | g(64,*)       -> [T, 6T]
NF = 2 * H + T
NGA = 7 * T
NGB = 4 * T
NGC = 6 * T


def _build_module():
    nc = bacc.Bacc(None, target_bir_lowering=False, debug=False)

    fa = nc.dram_tensor("fa", [T, NF], BF16, kind="ExternalInput")
    fb = nc.dram_tensor("fb", [T, NF], BF16, kind="ExternalInput")
    gad = nc.dram_tensor("ga", [T, NGA], BF16, kind="ExternalInput")
    gbd = nc.dram_tensor("gb", [T, NGB], BF16, kind="ExternalInput")
    gcd = nc.dram_tensor("gc", [T, NGC], BF16, kind="ExternalInput")
    out = nc.dram_tensor("out", [T, W], BF16, kind="ExternalOutput")

    with tile.TileContext(nc) as tc:
        with (
            tc.tile_pool(name="sb", bufs=1) as sbp,
            tc.tile_pool(name="acc", bufs=1, space=bass.MemorySpace.PSUM) as accp,
            tc.tile_pool(name="jk", bufs=1, space=bass.MemorySpace.PSUM) as jkp,
            tc.tile_pool(name="shp", bufs=1, space=bass.MemorySpace.PSUM) as shpp,
        ):
            fat = sbp.tile([T, NF], BF16, name="fat")
            fbt = sbp.tile([T, NF], BF16, name="fbt")
            ga = sbp.tile([T, NGA], BF16, name="ga")
            gb = sbp.tile([T, NGB], BF16, name="gb")
            gc = sbp.tile([T, NGC], BF16, name="gc")
            wz = sbp.tile([T, H], BF16, name="wz")

            # warmup weights first so the PE can start ramping immediately;
            # the DVE queue is empty until L2, so this runs right after the
            # entry barrier
            nc.vector.memset(wz[:, :], 0.0)

            nc.sync.dma_start(out=fat[:, :], in_=fa[:, :])
            nc.scalar.dma_start(out=fbt[:, :], in_=fb[:, :])
            nc.sync.dma_start(out=ga[:, :], in_=gad[:, :])
            nc.scalar.dma_start(out=gb[:, :], in_=gbd[:, :])
            nc.gpsimd.dma_start(out=gc[:, :], in_=gcd[:, :])

            lv = {}
            for s in SIZES[1:]:
                lv[s] = sbp.tile([ROWS[s], W], BF16, name=f"L{s}")

            def band_ap(s2):
                s = s2 // 2
                if s2 == 4:
                    return fat[0 : ROWS[s], 2 * H : 2 * H + ROWS[s2]]
                if s2 == 8:
                    return fbt[0 : ROWS[s], 2 * H : 2 * H + ROWS[s2]]
                if s2 == 16:
                    return ga[0 : ROWS[s], 4 * T : 4 * T + ROWS[s2]]
                if s2 == 32:
                    return ga[0 : ROWS[s], 5 * T : 5 * T + ROWS[s2]]
                return ga[0 : ROWS[s], 6 * T : 6 * T + ROWS[s2]]

            def g_ap(s, gi):
                if s == 1:
                    return ga[0 : ROWS[s], gi * T : (gi + 1) * T]
                if s == 2:
                    o = (2 + gi) * T
                    return ga[0 : ROWS[s], o : o + T]
                if s == 4:
                    o = gi * T
                    return gb[0 : ROWS[s], o : o + T]
                if s == 8:
                    o = (2 + gi) * T
                    return gb[0 : ROWS[s], o : o + T]
                if s == 16:
                    o = gi * T
                    return gc[0 : ROWS[s], o : o + T]
                if s == 32:
                    o = (2 + gi) * T
                    return gc[0 : ROWS[s], o : o + T]
                o = (4 + gi) * T
                return gc[0 : ROWS[s], o : o + T]

            acc = [accp.tile([T, W], F32, name=f"acc{gi}") for gi in range(2)]
            jk = jkp.tile([T, H], F32, name="jk")

            def junk(n):
                for _ in range(n):
                    nc.tensor.matmul(
                        jk[:, :], wz[0:T, 0:T], wz[0:T, 0:H],
                        start=True, stop=True,
                    )

            # warmup while the input DMAs land: full-width matmuls earn the
            # HAM full-clock promotion; count tuned to end as the first
            # ladder shift becomes ready (the early fills keep PE busy after)
            junk(7)

            # L2 halves from fa/fb as they arrive
            nc.vector.tensor_max(lv[2][:, 0:H], fat[0:125, 0:H], fat[0:125, H : 2 * H])
            nc.vector.tensor_max(lv[2][:, H:W], fbt[0:125, 0:H], fbt[0:125, H : 2 * H])

            first = {0: True, 1: True}

            def gather(s, gi, stop=False):
                for h in range(2):
                    if s == 1:
                        src = (fat if h == 0 else fbt)[0:126, 0:H]
                    else:
                        src = lv[s][0 : ROWS[s], h * H : (h + 1) * H]
                    nc.tensor.matmul(
                        acc[gi][:, h * H : (h + 1) * H],
                        g_ap(s, gi),
                        src,
                        start=first[gi],
                        stop=stop,
                    )
                first[gi] = False

            # each gap fills its own level's gathers (L_s is the shift's rhs,
            # so it's always ready); only s=64 remains after the ladder
            shift_plan = [
                (2, 4, [(1, 0), (1, 1), (2, 0)]),
                (4, 8, [(2, 1), (4, 0), (4, 1)]),
                (8, 16, [(8, 0), (8, 1)]),
                (16, 32, [(16, 0), (16, 1)]),
                (32, 64, [(32, 0), (32, 1)]),
            ]
            for s, s2, fills in shift_plan:
                ps = [
                    shpp.tile([T, H], F32, name=f"ps{s}h{h}", tag=f"psh{h}")
                    for h in range(2)
                ]
                for h in range(2):
                    nc.tensor.matmul(
                        ps[h][0 : ROWS[s2], :],
                        band_ap(s2),
                        lv[s][0 : ROWS[s], h * H : (h + 1) * H],
                        start=True,
                        stop=True,
                    )
                for fs, fgi in fills:
                    gather(fs, fgi)
                for h in range(2):
                    nc.vector.tensor_max(
                        lv[s2][:, h * H : (h + 1) * H],
                        lv[s][0 : ROWS[s2], h * H : (h + 1) * H],
                        ps[h][0 : ROWS[s2], :],
                    )

            gather(64, 0, stop=True)
            gather(64, 1, stop=True)

            mc = sbp.tile([T, W], F32, name="mc")
            ot = sbp.tile([T, W], BF16, name="ot")
            for h in range(2):
                sl = slice(h * H, (h + 1) * H)
                nc.scalar.copy(out=mc[:, sl], in_=acc[0][:, sl])
                nc.vector.tensor_max(ot[:, sl], mc[:, sl], acc[1][:, sl])
                eng = nc.sync if h == 0 else nc.scalar
                eng.dma_start(out=out[:, sl], in_=ot[:, sl])

    nc.compile()
    return nc


def _host_windows(segments):
    seg = np.clip(segments.astype(np.float32), 0.0, 125.0)
    row = seg[0]
    s0 = np.floor(row[:, 0]).astype(np.int32)
    s1 = np.ceil(row[:, 1]).astype(np.int32)
    s1 = np.where(s0 == s1, s1 + 1, s1)
    e0 = np.floor(row[:, 2]).astype(np.int32)
    e1 = np.ceil(row[:, 3]).astype(np.int32)
    e0 = np.where(e0 == e1, e0 - 1, e0)
    halves = []
    for lo, hi in ((s0, s1), (e0, e1)):
        lo_c = np.maximum(lo, 0)
        hi_c = np.minimum(hi, T)
        empty = lo_c >= hi_c
        halves.append((lo_c, hi_c, empty))
    return halves


def _family_onehots(lo, hi, empty):
    g = {(s, gi): np.zeros((ROWS[s], T), np.float32) for s in SIZES for gi in (0, 1)}
    for t in range(T):
        if empty[t]:
            continue
        ln = int(hi[t] - lo[t])
        s = 1 << (ln.bit_length() - 1)
        a = int(lo[t])
        b = int(hi[t]) - s
        g[(s, 0)][a, t] = 1.0
        g[(s, 1)][b, t] = 1.0
    return g


def _bands():
    out = {}
    for s2 in [4, 8, 16, 32, 64]:
        s = s2 // 2
        m = np.zeros((ROWS[s], ROWS[s2]), np.float32)
        for j2 in range(ROWS[s2]):
            m[j2 + s, j2] = 1.0
        out[s2] = m
    return out


def _to_bf16(x):
    import ml_dtypes

    return x.astype(ml_dtypes.bfloat16)


def _slab(m):
    s = np.zeros((T, T), np.float32)
    s[: m.shape[0], : m.shape[1]] = m
    return s


def _pack_family(g):
    bands = _bands()
    a = np.concatenate(
        [
            _slab(g[(1, 0)]),
            _slab(g[(1, 1)]),
            _slab(g[(2, 0)]),
            _slab(g[(2, 1)]),
            _slab(bands[16]),
            _slab(bands[32]),
            _slab(bands[64]),
        ],
        axis=1,
    )
    b = np.concatenate(
        [_slab(g[(4, 0)]), _slab(g[(4, 1)]), _slab(g[(8, 0)]), _slab(g[(8, 1)])],
        axis=1,
    )
    c = np.concatenate(
        [
            _slab(g[(16, 0)]),
            _slab(g[(16, 1)]),
            _slab(g[(32, 0)]),
            _slab(g[(32, 1)]),
            _slab(g[(64, 0)]),
            _slab(g[(64, 1)]),
        ],
        axis=1,
    )
    return _to_bf16(a), _to_bf16(b), _to_bf16(c)


def _core_fab(feature, core, g):
    fam = 0 if core < 4 else 1
    b0 = (core % 4) * BPC
    blk = feature[b0 : b0 + BPC, fam * C : (fam + 1) * C, :]  # [4, 256, T]
    ft = np.ascontiguousarray(blk.transpose(2, 0, 1).reshape(T, W))  # [j, c']
    f1 = np.zeros((T, W), np.float32)
    f1[0:125] = ft[1:126]
    bands = _bands()
    fa = np.concatenate([ft[:, 0:H], f1[:, 0:H], _slab(bands[4])], axis=1)
    fb = np.concatenate([ft[:, H:W], f1[:, H:W], _slab(bands[8])], axis=1)
    return _to_bf16(fa), _to_bf16(fb)


def kernel(feature, segments):
    global LAST_RESULTS
    feature = np.ascontiguousarray(feature, dtype=np.float32)
    segments = np.ascontiguousarray(segments, dtype=np.float32)

    if "nc" not in _CACHE:
        _CACHE["nc"] = _build_module()
    nc = _CACHE["nc"]

    halves = _host_windows(segments)
    fams = []
    for fam in range(2):
        lo, hi, empty = halves[fam]
        g = _family_onehots(lo, hi, empty)
        fams.append((g, _pack_family(g)))

    in_maps = []
    for core in range(NCORES):
        fam = 0 if core < 4 else 1
        g, (a, bpk, cpk) = fams[fam]
        fa, fb = _core_fab(feature, core, g)
        in_maps.append({"fa": fa, "fb": fb, "ga": a, "gb": bpk, "gc": cpk})

    res = run_bass_kernel_spmd(nc, in_maps, list(range(NCORES)), trace=TRACE)
    LAST_RESULTS = res

    out = np.empty((B, C2, T), np.float32)
    for core in range(NCORES):
        fam = 0 if core < 4 else 1
        b0 = (core % 4) * BPC
        r = np.asarray(res.results[core]["out"]).astype(np.float32)  # [T, W]
        arr = r.reshape(T, BPC, C).transpose(1, 2, 0)  # [b, c, t]
        out[b0 : b0 + BPC, fam * C : (fam + 1) * C, :] = arr

    neg = np.finfo(np.float32).min
    for h, (_, _, empty) in enumerate(halves):
        if empty.any():
            out[:, h * C : (h + 1) * C, empty] = neg
    return out
